# revision 9
# baseline (speedup 1.0000x reference)
"""Multi-head-attention (single-head, no scaling) Bass kernel for 8 trn2 cores.

Data-parallel over q rows: each core takes N/8 = 512 rows of q, replicates
k/v/weights, computes its [512, N] score block + softmax + output block.

Per-core pipeline:
  A)  transpose wq/wk on PE (f32, rounded copyback to f32r),
      project qpT = (q @ wq.T + b).T in fp32r
  B1) stream k in panels of 512 rows: transpose -> kT, project kpT panel,
      matmul scores into resident [512, N] f32 block
  B2) per 128-row tile: softmax (DVE max / ACT exp+rowsum / DVE normalize),
      DMA attn out, cast bf16 + PE-transpose -> attnT
  C)  transpose wv (bf16); stream v in panels: cast bf16, transpose -> vT,
      project vp (bf16, bias via broadcast add)
  D)  transpose wo (bf16); x1 = attn @ vp (bf16), transpose x1,
      out = x1 @ wo.T + b, DMA x out
"""

import numpy as np

try:
    import concourse.bass  # noqa: F401
except ImportError:
    import sys
    for _p in ("/opt/trn_rl_repo", "/root/.axon_site/_ro/trn_rl_repo"):
        if _p not in sys.path:
            sys.path.append(_p)

import concourse.bacc as bacc
import concourse.mybir as mybir
from concourse.bass_utils import run_bass_kernel_spmd
from concourse.tile import TileContext
from concourse.masks import make_identity

P = 128
N_CORES = 8

f32 = mybir.dt.float32
f32r = mybir.dt.float32r
bf16 = mybir.dt.float16  # fp16: same cost as bf16, 10-bit mantissa
EXP = mybir.ActivationFunctionType.Exp
COPY = mybir.ActivationFunctionType.Copy
IDENT = mybir.ActivationFunctionType.Identity


def build(N=4096, D=1024):
    """Build + compile the per-core Bass program."""
    B = N // N_CORES            # q rows per core
    DC = D // P                 # contraction chunks (d)
    IT = B // P                 # i (q-row) tiles per core
    PW = 512                    # panel width (k/v rows per panel)
    LPAN = N // PW              # l panels
    RT = PW // P                # row-subtiles per panel
    LT = N // P                 # l chunks
    FS = min(512, D)            # free-dim slice
    MS = D // FS                # slices of D

    nc = bacc.Bacc("TRN2", target_bir_lowering=False, debug=False,
                   num_devices=N_CORES)

    q_d = nc.dram_tensor("q", [B, D], f32, kind="ExternalInput")
    k_d = nc.dram_tensor("k", [N, D], f32, kind="ExternalInput")
    v_d = nc.dram_tensor("v", [N, D], f32, kind="ExternalInput")
    w_d = {}
    b_d = {}
    for nm in ("wq", "wk", "wv", "wo"):
        w_d[nm] = nc.dram_tensor(nm + "_w", [D, D], f32, kind="ExternalInput")
        b_d[nm] = nc.dram_tensor(nm + "_b", [D], f32, kind="ExternalInput")
    x_out = nc.dram_tensor("x_out", [B, D], f32, kind="ExternalOutput")
    a_out = nc.dram_tensor("attn_out", [B, N], f32, kind="ExternalOutput")

    def transpose_weight(nm, wT, sb, ps, out_dt):
        """DMA weight natural, PE-transpose 128x128 blocks into wT."""
        ident = ident32
        for jt in range(DC):
            wnat = sb.tile([P, D], f32, tag="wnat")
            nc.sync.dma_start(wnat[:], w_d[nm][jt * P:(jt + 1) * P, :])
            for dt in range(DC):
                pt = ps.tile([P, P], f32, tag="tpw")
                nc.tensor.transpose(pt[:], wnat[:, dt * P:(dt + 1) * P], ident[:])
                dst = wT[:, dt, jt * P:(jt + 1) * P]
                if out_dt is f32r:
                    nc.scalar.activation(dst.bitcast(f32r), pt[:], COPY)
                else:
                    nc.scalar.activation(dst, pt[:], COPY)

    with TileContext(nc) as tc:
        const_cm = tc.tile_pool(name="const", bufs=1)
        const = const_cm.__enter__()

        ident32 = const.tile([P, P], f32, tag="id32")
        make_identity(nc, ident32[:])
        ident16 = const.tile([P, P], bf16, tag="id16")
        make_identity(nc, ident16[:])

        qb_col = const.tile([P, DC], f32, tag="qbcol")
        kb_col = const.tile([P, DC], f32, tag="kbcol")
        for jt in range(DC):
            nc.sync.dma_start(qb_col[:, jt:jt + 1],
                              b_d["wq"][jt * P:(jt + 1) * P])
            nc.sync.dma_start(kb_col[:, jt:jt + 1],
                              b_d["wk"][jt * P:(jt + 1) * P])
        vb_bc = const.tile([P, D], f32, tag="vbbc")
        ob_bc = const.tile([P, D], f32, tag="obbc")
        nc.sync.dma_start(vb_bc[:],
                          b_d["wv"].rearrange("(o d) -> o d", o=1).partition_broadcast(P))
        nc.sync.dma_start(ob_bc[:],
                          b_d["wo"].rearrange("(o d) -> o d", o=1).partition_broadcast(P))

        # ---- phase A: wq/wk transposes + q projection ----------------
        poolAB_cm = tc.tile_pool(name="ab", bufs=1)  # wkT, qpT: A..B1
        poolAB = poolAB_cm.__enter__()
        wkT = poolAB.tile([P, DC, D], f32, tag="wkT")
        qpT = poolAB.tile([P, DC, B], f32, tag="qpT")

        with (
            tc.tile_pool(name="a_w", bufs=1) as a_w,
            tc.tile_pool(name="a_sb", bufs=3) as a_sb,
            tc.tile_pool(name="a_ps", bufs=3, space="PSUM") as a_ps,
            tc.tile_pool(name="a_ps2", bufs=2, space="PSUM") as a_ps2,
        ):
            wqT = a_w.tile([P, DC, D], f32, tag="wqT")
            transpose_weight("wq", wqT, a_sb, a_ps, f32r)
            transpose_weight("wk", wkT, a_sb, a_ps, f32r)

            qT = a_w.tile([P, DC, B], f32, tag="qT")
            for rt in range(IT):
                qnat = a_sb.tile([P, D], f32, tag="wnat")
                nc.sync.dma_start(qnat[:], q_d[rt * P:(rt + 1) * P, :])
                for dt in range(DC):
                    pt = a_ps.tile([P, P], f32, tag="tpw")
                    nc.tensor.transpose(pt[:], qnat[:, dt * P:(dt + 1) * P],
                                        ident32[:])
                    nc.vector.tensor_copy(
                        qT[:, dt, rt * P:(rt + 1) * P].bitcast(f32r), pt[:])
            for jt in range(DC):
                pj = a_ps2.tile([P, B], f32, tag="pq")
                for dt in range(DC):
                    nc.tensor.matmul(pj[:],
                                     wqT[:, dt, jt * P:(jt + 1) * P].bitcast(f32r),
                                     qT[:, dt, :].bitcast(f32r),
                                     start=(dt == 0), stop=(dt == DC - 1))
                nc.scalar.activation(qpT[:, jt, :].bitcast(f32r), pj[:], IDENT,
                                     bias=qb_col[:, jt:jt + 1])

        # ---- phase B1: k panels -> scores ----------------------------
        poolB_cm = tc.tile_pool(name="scores", bufs=1, side="right")  # scoresR: B1..B2
        poolB = poolB_cm.__enter__()
        scoresR = poolB.tile([P, IT, N], f32, tag="scores")

        with (
            tc.tile_pool(name="b_k", bufs=3) as b_k,
            tc.tile_pool(name="b_sb", bufs=1) as b_sb,
            tc.tile_pool(name="b_ps", bufs=2, space="PSUM") as b_ps,
            tc.tile_pool(name="b_ps2", bufs=2, space="PSUM") as b_ps2,
            tc.tile_pool(name="b_ps3", bufs=2, space="PSUM") as b_ps3,
        ):
            for lp in range(LPAN):
                kT = b_sb.tile([P, DC, PW], f32, tag="kT")
                for rt in range(RT):
                    knat = b_k.tile([P, D], f32, tag="knat")
                    nc.sync.dma_start(
                        knat[:], k_d[lp * PW + rt * P:lp * PW + (rt + 1) * P, :])
                    for dt in range(DC):
                        pt = b_ps.tile([P, P], f32, tag="tp")
                        nc.tensor.transpose(pt[:], knat[:, dt * P:(dt + 1) * P],
                                            ident32[:])
                        nc.vector.tensor_copy(
                            kT[:, dt, rt * P:(rt + 1) * P].bitcast(f32r), pt[:])
                kpT = b_sb.tile([P, DC, PW], f32, tag="kpT")
                for jt in range(DC):
                    pj = b_ps2.tile([P, PW], f32, tag="pk")
                    for dt in range(DC):
                        nc.tensor.matmul(pj[:],
                                         wkT[:, dt, jt * P:(jt + 1) * P].bitcast(f32r),
                                         kT[:, dt, :].bitcast(f32r),
                                         start=(dt == 0), stop=(dt == DC - 1))
                    nc.scalar.activation(kpT[:, jt, :].bitcast(f32r), pj[:], IDENT,
                                         bias=kb_col[:, jt:jt + 1])
                for it in range(IT):
                    psc = b_ps3.tile([P, PW], f32, tag="ps")
                    for jt in range(DC):
                        nc.tensor.matmul(psc[:],
                                         qpT[:, jt, it * P:(it + 1) * P].bitcast(f32r),
                                         kpT[:, jt, :].bitcast(f32r),
                                         start=(jt == 0), stop=(jt == DC - 1))
                    nc.vector.tensor_copy(scoresR[:, it, lp * PW:(lp + 1) * PW],
                                          psc[:])
        poolAB_cm.__exit__(None, None, None)   # free wkT, qpT

        # ---- phase C: wv transpose + v panels -> vp (bf16) -----------
        poolCD_cm = tc.tile_pool(name="vp", bufs=1)  # vp: C..D
        poolCD = poolCD_cm.__enter__()
        vp = poolCD.tile([P, LT, D], bf16, tag="vp")

        with (
            tc.tile_pool(name="c_w", bufs=1) as c_w,
            tc.tile_pool(name="c_v", bufs=3) as c_v,
            tc.tile_pool(name="c_sb", bufs=1) as c_sb,
            tc.tile_pool(name="c_ps", bufs=3, space="PSUM") as c_ps,
            tc.tile_pool(name="c_ps2", bufs=2, space="PSUM") as c_ps2,
        ):
            wvT = c_w.tile([P, DC, D], bf16, tag="wvT")
            transpose_weight("wv", wvT, c_v, c_ps, bf16)
            for lp in range(LPAN):
                vT = c_sb.tile([P, DC, PW], bf16, tag="vT")
                for rt in range(RT):
                    vnat = c_v.tile([P, D], f32, tag="wnat")
                    nc.sync.dma_start(
                        vnat[:], v_d[lp * PW + rt * P:lp * PW + (rt + 1) * P, :])
                    vb = c_v.tile([P, D], bf16, tag="vb16")
                    nc.vector.tensor_copy(vb[:], vnat[:])
                    for dt in range(DC):
                        pt = c_ps.tile([P, P], bf16, tag="tp16")
                        nc.tensor.transpose(pt[:], vb[:, dt * P:(dt + 1) * P],
                                            ident16[:])
                        nc.vector.tensor_copy(vT[:, dt, rt * P:(rt + 1) * P],
                                              pt[:])
                for lc in range(RT):
                    for ms in range(MS):
                        pv = c_ps2.tile([P, FS], f32, tag="pv")
                        for dt in range(DC):
                            nc.tensor.matmul(pv[:], vT[:, dt, lc * P:(lc + 1) * P],
                                             wvT[:, dt, ms * FS:(ms + 1) * FS],
                                             start=(dt == 0), stop=(dt == DC - 1))
                        nc.vector.tensor_add(
                            vp[:, lp * RT + lc, ms * FS:(ms + 1) * FS],
                            pv[:], vb_bc[:, ms * FS:(ms + 1) * FS])

        # ---- phase B2: softmax + attn out + attnT --------------------
        poolBD_cm = tc.tile_pool(name="attnT", bufs=1)  # attnT: B2..D
        poolBD = poolBD_cm.__enter__()
        attnT = poolBD.tile([P, LT, B], bf16, tag="attnT")

        with (
            tc.tile_pool(name="s_sb", bufs=2) as s_sb,
            tc.tile_pool(name="s_ps", bufs=4, space="PSUM") as s_ps,
        ):
            for it in range(IT):
                sc = scoresR[:, it, :]
                nmax = s_sb.tile([P, 1], f32, tag="nmax")
                nc.vector.reduce_max(nmax[:], sc, axis=mybir.AxisListType.X,
                                     negate=True)
                zsum = s_sb.tile([P, 1], f32, tag="zsum")
                nc.scalar.activation(sc, sc, EXP, bias=nmax[:], scale=1.0,
                                     accum_out=zsum[:])
                zinv = s_sb.tile([P, 1], f32, tag="zinv")
                nc.vector.reciprocal(zinv[:], zsum[:])
                nc.vector.tensor_scalar_mul(sc, sc, zinv[:])
                nc.sync.dma_start(a_out[it * P:(it + 1) * P, :], sc)
                ab = s_sb.tile([P, N], bf16, tag="abf")
                nc.scalar.activation(ab[:], sc, COPY)
                for lt in range(LT):
                    pt = s_ps.tile([P, P], bf16, tag="tp16")
                    nc.tensor.transpose(pt[:], ab[:, lt * P:(lt + 1) * P],
                                        ident16[:])
                    nc.vector.tensor_copy(attnT[:, lt, it * P:(it + 1) * P], pt[:])
        poolB_cm.__exit__(None, None, None)    # free scoresR

        # ---- phase D: x1 = attn @ vp, out = x1 @ wo.T + b ------------
        with (
            tc.tile_pool(name="d_w", bufs=1) as d_w,
            tc.tile_pool(name="d_wn", bufs=3) as d_wn,
            tc.tile_pool(name="d_sb", bufs=2) as d_sb,
            tc.tile_pool(name="d_ps", bufs=2, space="PSUM") as d_ps,
            tc.tile_pool(name="d_psw", bufs=2, space="PSUM") as d_psw,
            tc.tile_pool(name="d_ps2", bufs=2, space="PSUM") as d_ps2,
        ):
            woT = d_w.tile([P, DC, D], bf16, tag="woT")
            transpose_weight("wo", woT, d_wn, d_psw, bf16)
            for it in range(IT):
                x1b = d_sb.tile([P, D], bf16, tag="x1b")
                for ms in range(MS):
                    px = d_ps.tile([P, FS], f32, tag="px")
                    for lt in range(LT):
                        nc.tensor.matmul(px[:], attnT[:, lt, it * P:(it + 1) * P],
                                         vp[:, lt, ms * FS:(ms + 1) * FS],
                                         start=(lt == 0), stop=(lt == LT - 1))
                    nc.scalar.activation(x1b[:, ms * FS:(ms + 1) * FS], px[:],
                                         COPY)
                x1T = d_sb.tile([P, DC, P], bf16, tag="x1T")
                for mt in range(DC):
                    pt = d_ps.tile([P, P], bf16, tag="tp16d")
                    nc.tensor.transpose(pt[:], x1b[:, mt * P:(mt + 1) * P],
                                        ident16[:])
                    nc.vector.tensor_copy(x1T[:, mt, :], pt[:])
                xo = d_sb.tile([P, D], f32, tag="xo")
                for cs in range(MS):
                    pxo = d_ps2.tile([P, FS], f32, tag="pxo")
                    for mt in range(DC):
                        nc.tensor.matmul(pxo[:], x1T[:, mt, :],
                                         woT[:, mt, cs * FS:(cs + 1) * FS],
                                         start=(mt == 0), stop=(mt == DC - 1))
                    nc.vector.tensor_add(xo[:, cs * FS:(cs + 1) * FS], pxo[:],
                                         ob_bc[:, cs * FS:(cs + 1) * FS])
                nc.sync.dma_start(x_out[it * P:(it + 1) * P, :], xo[:])

        poolBD_cm.__exit__(None, None, None)
        poolCD_cm.__exit__(None, None, None)
        const_cm.__exit__(None, None, None)

    nc.compile()
    return nc


_built = {}


def _get_nc(N=4096, D=1024):
    key = (N, D)
    if key not in _built:
        _built[key] = build(N, D)
    return _built[key]


def kernel(**inputs):
    q = np.ascontiguousarray(np.asarray(inputs["q"], dtype=np.float32))
    k = np.ascontiguousarray(np.asarray(inputs["k"], dtype=np.float32))
    v = np.ascontiguousarray(np.asarray(inputs["v"], dtype=np.float32))
    N, D = k.shape
    B = N // N_CORES
    nc = _get_nc(N, D)
    shared = {"k": k, "v": v}
    for nm in ("wq", "wk", "wv", "wo"):
        shared[nm + "_w"] = np.ascontiguousarray(
            np.asarray(inputs[nm + "_w"], dtype=np.float32))
        shared[nm + "_b"] = np.ascontiguousarray(
            np.asarray(inputs[nm + "_b"], dtype=np.float32))
    in_maps = [dict(shared, q=q[c * B:(c + 1) * B]) for c in range(N_CORES)]
    res = run_bass_kernel_spmd(nc, in_maps, core_ids=list(range(N_CORES)))
    x = np.concatenate([res.results[c]["x_out"] for c in range(N_CORES)], axis=0)
    attn = np.concatenate([res.results[c]["attn_out"] for c in range(N_CORES)],
                          axis=0)
    return (x, attn)


# revision 11
# speedup vs baseline: 1.2523x; 1.2523x over previous
"""Multi-head-attention (single-head, no scaling) Bass kernel for 8 trn2 cores.

v2: distributed K/V projections + AllGather.

Sharding: core c owns q rows [c*B, (c+1)*B) AND k/v rows [c*B, (c+1)*B).
Weights replicated. Each core:
  A)  wk transpose (PE, f32 -> rounded f32r copyback), own-k transpose,
      kpT_own = (wk @ k_own.T + b) [D, B] f32r -> DRAM -> AllGather (CC queue)
      wq transpose, qT, qpT (overlaps the AllGather)
  C)  wv (fp16) transpose, own-v cast+transpose, vp_own [B, D] fp16
      (+bias) -> DRAM -> AllGather; wo (fp16) transpose
  B1) for each 512-panel: DMA gathered kpT panel, DVE re-round to f32r,
      scores matmuls -> resident [B, N] f32
  B2) per 128-row tile: softmax (DVE/ACT, fused exp+rowsum), DMA attn out,
      cast fp16 + PE transpose -> attnT
  Dv) DMA gathered vp -> SBUF fp16
  D)  x1 = attn @ vp, transpose, x = x1 @ wo.T + b -> DMA out
"""

import numpy as np

try:
    import concourse.bass  # noqa: F401
except ImportError:
    import sys
    for _p in ("/opt/trn_rl_repo", "/root/.axon_site/_ro/trn_rl_repo"):
        if _p not in sys.path:
            sys.path.append(_p)

import concourse.bacc as bacc
import concourse.mybir as mybir
from concourse.bass_utils import run_bass_kernel_spmd
from concourse.tile import TileContext
from concourse.masks import make_identity

P = 128
N_CORES = 8

f32 = mybir.dt.float32
f32r = mybir.dt.float32r
f16 = mybir.dt.float16
EXP = mybir.ActivationFunctionType.Exp
COPY = mybir.ActivationFunctionType.Copy
IDENT = mybir.ActivationFunctionType.Identity


def build(N=4096, D=1024):
    B = N // N_CORES            # q/k/v rows per core
    DC = D // P                 # contraction chunks (d)
    IT = B // P                 # 128-row tiles per core slice
    PW = B                      # scores panel width == per-core slice
    LPAN = N // PW
    LT = N // P
    FS = min(512, D)
    MS = D // FS

    nc = bacc.Bacc("TRN2", target_bir_lowering=False, debug=False,
                   num_devices=N_CORES)

    q_d = nc.dram_tensor("q", [B, D], f32, kind="ExternalInput")
    k_d = nc.dram_tensor("k", [B, D], f32, kind="ExternalInput")
    v_d = nc.dram_tensor("v", [B, D], f32, kind="ExternalInput")
    w_d = {}
    b_d = {}
    for nm in ("wq", "wk", "wv", "wo"):
        w_d[nm] = nc.dram_tensor(nm + "_w", [D, D], f32, kind="ExternalInput")
        b_d[nm] = nc.dram_tensor(nm + "_b", [D], f32, kind="ExternalInput")
    x_out = nc.dram_tensor("x_out", [B, D], f32, kind="ExternalOutput")
    a_out = nc.dram_tensor("attn_out", [B, N], f32, kind="ExternalOutput")

    with TileContext(nc) as tc:
        const_cm = tc.tile_pool(name="const", bufs=1)
        const = const_cm.__enter__()
        dram_cm = tc.tile_pool(name="dram", bufs=1, space="DRAM")
        dram = dram_cm.__enter__()

        kp_in = dram.tile([D, B], f32, tag="kp_in")
        kp_all = dram.tile([N_CORES, D, B], f32, tag="kp_all")
        vp_in = dram.tile([B, D], f16, tag="vp_in")
        vp_all = dram.tile([N_CORES, B, D], f16, tag="vp_all")

        ident32 = const.tile([P, P], f32, tag="id32")
        make_identity(nc, ident32[:])
        ident16 = const.tile([P, P], f16, tag="id16")
        make_identity(nc, ident16[:])

        qb_col = const.tile([P, DC], f32, tag="qbcol")
        kb_col = const.tile([P, DC], f32, tag="kbcol")
        for jt in range(DC):
            nc.sync.dma_start(qb_col[:, jt:jt + 1],
                              b_d["wq"][jt * P:(jt + 1) * P])
            nc.sync.dma_start(kb_col[:, jt:jt + 1],
                              b_d["wk"][jt * P:(jt + 1) * P])
        vb_bc = const.tile([P, D], f32, tag="vbbc")
        ob_bc = const.tile([P, D], f32, tag="obbc")
        nc.sync.dma_start(vb_bc[:],
                          b_d["wv"].rearrange("(o d) -> o d", o=1).partition_broadcast(P))
        nc.sync.dma_start(ob_bc[:],
                          b_d["wo"].rearrange("(o d) -> o d", o=1).partition_broadcast(P))

        def transpose_weight(nm, wT, sb, ps, rnd):
            """DMA weight natural, PE-transpose 128x128 blocks into wT."""
            for jt in range(DC):
                wnat = sb.tile([P, D], f32, tag="wnat")
                nc.sync.dma_start(wnat[:], w_d[nm][jt * P:(jt + 1) * P, :])
                for dt in range(DC):
                    pt = ps.tile([P, P], f32, tag="tpw")
                    nc.tensor.transpose(pt[:], wnat[:, dt * P:(dt + 1) * P],
                                        ident32[:])
                    dst = wT[:, dt, jt * P:(jt + 1) * P]
                    if rnd is f32r:
                        nc.scalar.activation(dst.bitcast(f32r), pt[:], COPY)
                    else:
                        nc.scalar.activation(dst, pt[:], COPY)

        def transpose_rows(src_d, rows, dst, sb, ps, dt_out):
            """DMA rows of src_d, transpose into dst [P, DC, rows]."""
            for rt in range(rows // P):
                nat = sb.tile([P, D], f32, tag="wnat")
                nc.sync.dma_start(nat[:], src_d[rt * P:(rt + 1) * P, :])
                if dt_out is f16:
                    c16 = sb.tile([P, D], f16, tag="nat16")
                    nc.vector.tensor_copy(c16[:], nat[:])
                    for dt in range(DC):
                        pt = ps.tile([P, P], f16, tag="tp16")
                        nc.tensor.transpose(pt[:], c16[:, dt * P:(dt + 1) * P],
                                            ident16[:])
                        nc.vector.tensor_copy(dst[:, dt, rt * P:(rt + 1) * P],
                                              pt[:])
                else:
                    for dt in range(DC):
                        pt = ps.tile([P, P], f32, tag="tpw")
                        nc.tensor.transpose(pt[:], nat[:, dt * P:(dt + 1) * P],
                                            ident32[:])
                        nc.vector.tensor_copy(
                            dst[:, dt, rt * P:(rt + 1) * P].bitcast(f32r), pt[:])

        # ---- phase A: kpT_own -> AllGather; qpT ----------------------
        poolA_cm = tc.tile_pool(name="qpT", bufs=1)   # qpT: A..B1
        poolA = poolA_cm.__enter__()
        qpT = poolA.tile([P, DC, B], f32, tag="qpT")

        with (
            tc.tile_pool(name="a_w", bufs=1) as a_w,
            tc.tile_pool(name="a_sb", bufs=3) as a_sb,
            tc.tile_pool(name="a_ps", bufs=3, space="PSUM") as a_ps,
            tc.tile_pool(name="a_ps2", bufs=2, space="PSUM") as a_ps2,
        ):
            # --- k path first so the AllGather launches early ---
            wkT = a_w.tile([P, DC, D], f32, tag="wkT")
            transpose_weight("wk", wkT, a_sb, a_ps, f32r)
            kTo = a_w.tile([P, DC, B], f32, tag="kTo")
            transpose_rows(k_d, B, kTo, a_sb, a_ps, f32r)
            kpTo = a_w.tile([P, DC, B], f32, tag="kpTo")
            for jt in range(DC):
                pj = a_ps2.tile([P, B], f32, tag="pq")
                for dt in range(DC):
                    nc.tensor.matmul(pj[:],
                                     wkT[:, dt, jt * P:(jt + 1) * P].bitcast(f32r),
                                     kTo[:, dt, :].bitcast(f32r),
                                     start=(dt == 0), stop=(dt == DC - 1))
                nc.scalar.activation(kpTo[:, jt, :].bitcast(f32r), pj[:], IDENT,
                                     bias=kb_col[:, jt:jt + 1])
                nc.sync.dma_start(kp_in[jt * P:(jt + 1) * P, :], kpTo[:, jt, :])
            nc.gpsimd.collective_compute(
                "AllGather", mybir.AluOpType.bypass,
                replica_groups=[list(range(N_CORES))],
                ins=[kp_in[:].opt()], outs=[kp_all[:].opt()])

            # --- q path (overlaps the gather) ---
            wqT = a_w.tile([P, DC, D], f32, tag="wqT")
            transpose_weight("wq", wqT, a_sb, a_ps, f32r)
            qT = a_w.tile([P, DC, B], f32, tag="qT")
            transpose_rows(q_d, B, qT, a_sb, a_ps, f32r)
            for jt in range(DC):
                pj = a_ps2.tile([P, B], f32, tag="pq")
                for dt in range(DC):
                    nc.tensor.matmul(pj[:],
                                     wqT[:, dt, jt * P:(jt + 1) * P].bitcast(f32r),
                                     qT[:, dt, :].bitcast(f32r),
                                     start=(dt == 0), stop=(dt == DC - 1))
                nc.scalar.activation(qpT[:, jt, :].bitcast(f32r), pj[:], IDENT,
                                     bias=qb_col[:, jt:jt + 1])

        # ---- phase C: vp_own -> AllGather; woT -----------------------
        poolC_cm = tc.tile_pool(name="woT", bufs=1, side="right")   # woT: C..D
        poolC = poolC_cm.__enter__()
        woT = poolC.tile([P, DC, D], f16, tag="woT")

        with (
            tc.tile_pool(name="c_w", bufs=1) as c_w,
            tc.tile_pool(name="c_sb", bufs=3) as c_sb,
            tc.tile_pool(name="c_ps", bufs=3, space="PSUM") as c_ps,
            tc.tile_pool(name="c_ps2", bufs=2, space="PSUM") as c_ps2,
        ):
            wvT = c_w.tile([P, DC, D], f16, tag="wvT")
            transpose_weight("wv", wvT, c_sb, c_ps, f16)
            vTo = c_w.tile([P, DC, B], f16, tag="vTo")
            transpose_rows(v_d, B, vTo, c_sb, c_ps, f16)
            vpo = c_w.tile([P, IT, D], f16, tag="vpo")
            for lc in range(IT):
                for ms in range(MS):
                    pv = c_ps2.tile([P, FS], f32, tag="pv")
                    for dt in range(DC):
                        nc.tensor.matmul(pv[:], vTo[:, dt, lc * P:(lc + 1) * P],
                                         wvT[:, dt, ms * FS:(ms + 1) * FS],
                                         start=(dt == 0), stop=(dt == DC - 1))
                    nc.vector.tensor_add(vpo[:, lc, ms * FS:(ms + 1) * FS],
                                         pv[:], vb_bc[:, ms * FS:(ms + 1) * FS])
                nc.sync.dma_start(vp_in[lc * P:(lc + 1) * P, :], vpo[:, lc, :])
            nc.gpsimd.collective_compute(
                "AllGather", mybir.AluOpType.bypass,
                replica_groups=[list(range(N_CORES))],
                ins=[vp_in[:].opt()], outs=[vp_all[:].opt()])

            # wo transposes fill the gather wait
            transpose_weight("wo", woT, c_sb, c_ps, f16)

        # ---- phase B1: gathered kpT panels -> scores -----------------
        poolB_cm = tc.tile_pool(name="scores", bufs=1, side="right")
        poolB = poolB_cm.__enter__()
        scoresR = poolB.tile([P, IT, N], f32, tag="scores")

        with (
            tc.tile_pool(name="b_sb", bufs=2) as b_sb,
            tc.tile_pool(name="b_ps3", bufs=4, space="PSUM") as b_ps3,
        ):
            for lp in range(LPAN):
                kpP = b_sb.tile([P, DC, PW], f32, tag="kpP")
                nc.sync.dma_start(
                    kpP[:], kp_all[lp].rearrange("(c p) b -> p c b", p=P))
                kpPr = b_sb.tile([P, DC, PW], f32, tag="kpPr")
                nc.vector.tensor_copy(kpPr[:].bitcast(f32r), kpP[:])
                for it in range(IT):
                    psc = b_ps3.tile([P, PW], f32, tag="ps")
                    for jt in range(DC):
                        nc.tensor.matmul(psc[:],
                                         qpT[:, jt, it * P:(it + 1) * P].bitcast(f32r),
                                         kpPr[:, jt, :].bitcast(f32r),
                                         start=(jt == 0), stop=(jt == DC - 1))
                    nc.vector.tensor_copy(scoresR[:, it, lp * PW:(lp + 1) * PW],
                                          psc[:])
        poolA_cm.__exit__(None, None, None)   # free qpT

        # ---- phase B2 + D: softmax, attnT, vp load, x1, out ----------
        poolBD_cm = tc.tile_pool(name="attnT", bufs=1)
        poolBD = poolBD_cm.__enter__()
        attnT = poolBD.tile([P, LT, B], f16, tag="attnT")
        vp = poolBD.tile([P, LT, D], f16, tag="vp")

        # vp: DMA gathered [N_CORES, B, D] -> [P, LT, D]
        for lc in range(LT):
            c, r = divmod(lc, IT)
            nc.sync.dma_start(vp[:, lc, :], vp_all[c, r * P:(r + 1) * P, :])

        with (
            tc.tile_pool(name="s_sb", bufs=2) as s_sb,
            tc.tile_pool(name="s_ps", bufs=4, space="PSUM") as s_ps,
        ):
            for it in range(IT):
                sc = scoresR[:, it, :]
                nmax = s_sb.tile([P, 1], f32, tag="nmax")
                nc.vector.reduce_max(nmax[:], sc, axis=mybir.AxisListType.X,
                                     negate=True)
                zsum = s_sb.tile([P, 1], f32, tag="zsum")
                nc.scalar.activation(sc, sc, EXP, bias=nmax[:], scale=1.0,
                                     accum_out=zsum[:])
                zinv = s_sb.tile([P, 1], f32, tag="zinv")
                nc.vector.reciprocal(zinv[:], zsum[:])
                nc.vector.tensor_scalar_mul(sc, sc, zinv[:])
                ab = s_sb.tile([P, N], f16, tag="abf")
                nc.scalar.activation(ab[:], sc, COPY)
                for lt in range(LT):
                    pt = s_ps.tile([P, P], f16, tag="tp16")
                    nc.tensor.transpose(pt[:], ab[:, lt * P:(lt + 1) * P],
                                        ident16[:])
                    nc.vector.tensor_copy(attnT[:, lt, it * P:(it + 1) * P],
                                          pt[:])
                nc.sync.dma_start(a_out[it * P:(it + 1) * P, :], sc)
        poolB_cm.__exit__(None, None, None)   # free scoresR

        with (
            tc.tile_pool(name="d_sb", bufs=2) as d_sb,
            tc.tile_pool(name="d_ps", bufs=2, space="PSUM") as d_ps,
            tc.tile_pool(name="d_ps2", bufs=2, space="PSUM") as d_ps2,
        ):
            for it in range(IT):
                x1b = d_sb.tile([P, D], f16, tag="x1b")
                for ms in range(MS):
                    px = d_ps.tile([P, FS], f32, tag="px")
                    for lt in range(LT):
                        nc.tensor.matmul(px[:], attnT[:, lt, it * P:(it + 1) * P],
                                         vp[:, lt, ms * FS:(ms + 1) * FS],
                                         start=(lt == 0), stop=(lt == LT - 1))
                    nc.scalar.activation(x1b[:, ms * FS:(ms + 1) * FS], px[:],
                                         COPY)
                x1T = d_sb.tile([P, DC, P], f16, tag="x1T")
                for mt in range(DC):
                    pt = d_ps.tile([P, P], f16, tag="tp16d")
                    nc.tensor.transpose(pt[:], x1b[:, mt * P:(mt + 1) * P],
                                        ident16[:])
                    nc.vector.tensor_copy(x1T[:, mt, :], pt[:])
                xo = d_sb.tile([P, D], f32, tag="xo")
                for cs in range(MS):
                    pxo = d_ps2.tile([P, FS], f32, tag="pxo")
                    for mt in range(DC):
                        nc.tensor.matmul(pxo[:], x1T[:, mt, :],
                                         woT[:, mt, cs * FS:(cs + 1) * FS],
                                         start=(mt == 0), stop=(mt == DC - 1))
                    nc.vector.tensor_add(xo[:, cs * FS:(cs + 1) * FS], pxo[:],
                                         ob_bc[:, cs * FS:(cs + 1) * FS])
                nc.sync.dma_start(x_out[it * P:(it + 1) * P, :], xo[:])

        poolBD_cm.__exit__(None, None, None)
        poolC_cm.__exit__(None, None, None)
        dram_cm.__exit__(None, None, None)
        const_cm.__exit__(None, None, None)

    nc.compile()
    return nc


_built = {}


def _get_nc(N=4096, D=1024):
    key = (N, D)
    if key not in _built:
        _built[key] = build(N, D)
    return _built[key]


def _make_in_maps(inputs):
    q = np.ascontiguousarray(np.asarray(inputs["q"], dtype=np.float32))
    k = np.ascontiguousarray(np.asarray(inputs["k"], dtype=np.float32))
    v = np.ascontiguousarray(np.asarray(inputs["v"], dtype=np.float32))
    N, D = k.shape
    B = N // N_CORES
    shared = {}
    for nm in ("wq", "wk", "wv", "wo"):
        shared[nm + "_w"] = np.ascontiguousarray(
            np.asarray(inputs[nm + "_w"], dtype=np.float32))
        shared[nm + "_b"] = np.ascontiguousarray(
            np.asarray(inputs[nm + "_b"], dtype=np.float32))
    return [dict(shared,
                 q=q[c * B:(c + 1) * B],
                 k=np.ascontiguousarray(k[c * B:(c + 1) * B]),
                 v=np.ascontiguousarray(v[c * B:(c + 1) * B]))
            for c in range(N_CORES)], N, D


def kernel(**inputs):
    in_maps, N, D = _make_in_maps(inputs)
    nc = _get_nc(N, D)
    res = run_bass_kernel_spmd(nc, in_maps, core_ids=list(range(N_CORES)))
    x = np.concatenate([res.results[c]["x_out"] for c in range(N_CORES)], axis=0)
    attn = np.concatenate([res.results[c]["attn_out"] for c in range(N_CORES)],
                          axis=0)
    return (x, attn)


# revision 14
# speedup vs baseline: 1.2828x; 1.0244x over previous
"""Multi-head-attention (single-head, no scaling) Bass kernel for 8 trn2 cores.

v2: distributed K/V projections + AllGather.

Sharding: core c owns q rows [c*B, (c+1)*B) AND k/v rows [c*B, (c+1)*B).
Weights replicated. Each core:
  A)  wk transpose (PE, f32 -> rounded f32r copyback), own-k transpose,
      kpT_own = (wk @ k_own.T + b) [D, B] f32r -> DRAM -> AllGather (CC queue)
      wq transpose, qT, qpT (overlaps the AllGather)
  C)  wv (fp16) transpose, own-v cast+transpose, vp_own [B, D] fp16
      (+bias) -> DRAM -> AllGather; wo (fp16) transpose
  B1) for each 512-panel: DMA gathered kpT panel, DVE re-round to f32r,
      scores matmuls -> resident [B, N] f32
  B2) per 128-row tile: softmax (DVE/ACT, fused exp+rowsum), DMA attn out,
      cast fp16 + PE transpose -> attnT
  Dv) DMA gathered vp -> SBUF fp16
  D)  x1 = attn @ vp, transpose, x = x1 @ wo.T + b -> DMA out
"""

import numpy as np

_DIRECT_F32R = True

try:
    import concourse.bass  # noqa: F401
except ImportError:
    import sys
    for _p in ("/opt/trn_rl_repo", "/root/.axon_site/_ro/trn_rl_repo"):
        if _p not in sys.path:
            sys.path.append(_p)

import concourse.bacc as bacc
import concourse.mybir as mybir
from concourse.bass_utils import run_bass_kernel_spmd
from concourse.tile import TileContext
from concourse.masks import make_identity

P = 128
N_CORES = 8

f32 = mybir.dt.float32
f32r = mybir.dt.float32r
f16 = mybir.dt.float16
EXP = mybir.ActivationFunctionType.Exp
COPY = mybir.ActivationFunctionType.Copy
IDENT = mybir.ActivationFunctionType.Identity


def build(N=4096, D=1024):
    B = N // N_CORES            # q/k/v rows per core
    DC = D // P                 # contraction chunks (d)
    IT = B // P                 # 128-row tiles per core slice
    PW = B                      # scores panel width == per-core slice
    LPAN = N // PW
    LT = N // P
    FS = min(512, D)
    MS = D // FS

    nc = bacc.Bacc("TRN2", target_bir_lowering=False, debug=False,
                   num_devices=N_CORES)

    q_d = nc.dram_tensor("q", [B, D], f32, kind="ExternalInput")
    k_d = nc.dram_tensor("k", [B, D], f32, kind="ExternalInput")
    v_d = nc.dram_tensor("v", [B, D], f32, kind="ExternalInput")
    w_d = {}
    b_d = {}
    for nm in ("wq", "wk", "wv", "wo"):
        w_d[nm] = nc.dram_tensor(nm + "_w", [D, D], f32, kind="ExternalInput")
        b_d[nm] = nc.dram_tensor(nm + "_b", [D], f32, kind="ExternalInput")
    x_out = nc.dram_tensor("x_out", [B, D], f32, kind="ExternalOutput")
    a_out = nc.dram_tensor("attn_out", [B, N], f32, kind="ExternalOutput")

    with TileContext(nc) as tc:
        const_cm = tc.tile_pool(name="const", bufs=1)
        const = const_cm.__enter__()
        dram_cm = tc.tile_pool(name="dram", bufs=1, space="DRAM")
        dram = dram_cm.__enter__()

        kp_in = dram.tile([D, B], f32, tag="kp_in")
        kp_all = dram.tile([N_CORES, D, B], f32, tag="kp_all")
        vp_in = dram.tile([B, D], f16, tag="vp_in")
        vp_all = dram.tile([N_CORES, B, D], f16, tag="vp_all")

        ident32 = const.tile([P, P], f32, tag="id32")
        make_identity(nc, ident32[:])
        ident16 = const.tile([P, P], f16, tag="id16")
        make_identity(nc, ident16[:])

        qb_col = const.tile([P, DC], f32, tag="qbcol")
        kb_col = const.tile([P, DC], f32, tag="kbcol")
        for jt in range(DC):
            nc.sync.dma_start(qb_col[:, jt:jt + 1],
                              b_d["wq"][jt * P:(jt + 1) * P])
            nc.sync.dma_start(kb_col[:, jt:jt + 1],
                              b_d["wk"][jt * P:(jt + 1) * P])
        vb_bc = const.tile([P, D], f16, tag="vbbc")
        ob_bc = const.tile([P, D], f16, tag="obbc")
        with tc.tile_pool(name="btmp", bufs=2) as btmp:
            for bc_t, bnm in ((vb_bc, "wv"), (ob_bc, "wo")):
                tmp = btmp.tile([P, D], f32, tag="btmp")
                nc.sync.dma_start(
                    tmp[:],
                    b_d[bnm].rearrange("(o d) -> o d", o=1).partition_broadcast(P))
                nc.vector.tensor_copy(bc_t[:], tmp[:])

        def transpose_weight(nm, wT, sb, ps, rnd):
            """DMA weight natural, PE-transpose 128x128 blocks into wT."""
            for jt in range(DC):
                wnat = sb.tile([P, D], f32, tag="wnat")
                nc.sync.dma_start(wnat[:], w_d[nm][jt * P:(jt + 1) * P, :])
                for dt in range(DC):
                    pt = ps.tile([P, P], f32, tag="tpw")
                    nc.tensor.transpose(pt[:], wnat[:, dt * P:(dt + 1) * P],
                                        ident32[:])
                    dst = wT[:, dt, jt * P:(jt + 1) * P]
                    if rnd is f32r:
                        nc.scalar.activation(dst.bitcast(f32r), pt[:], COPY)
                    else:
                        nc.scalar.activation(dst, pt[:], COPY)

        def transpose_rows(src_d, rows, dst, sb, ps, dt_out):
            """DMA rows of src_d, transpose into dst [P, DC, rows]."""
            for rt in range(rows // P):
                nat = sb.tile([P, D], f32, tag="wnat")
                nc.sync.dma_start(nat[:], src_d[rt * P:(rt + 1) * P, :])
                if dt_out is f16:
                    c16 = sb.tile([P, D], f16, tag="nat16")
                    nc.vector.tensor_copy(c16[:], nat[:])
                    for dt in range(DC):
                        pt = ps.tile([P, P], f16, tag="tp16")
                        nc.tensor.transpose(pt[:], c16[:, dt * P:(dt + 1) * P],
                                            ident16[:])
                        nc.vector.tensor_copy(dst[:, dt, rt * P:(rt + 1) * P],
                                              pt[:])
                else:
                    for dt in range(DC):
                        pt = ps.tile([P, P], f32, tag="tpw")
                        nc.tensor.transpose(pt[:], nat[:, dt * P:(dt + 1) * P],
                                            ident32[:])
                        nc.vector.tensor_copy(
                            dst[:, dt, rt * P:(rt + 1) * P].bitcast(f32r), pt[:])

        # ---- phase A: kpT_own -> AllGather; qpT ----------------------
        poolA_cm = tc.tile_pool(name="qpT", bufs=1)   # qpT: A..B1
        poolA = poolA_cm.__enter__()
        qpT = poolA.tile([P, DC, B], f32, tag="qpT")

        with (
            tc.tile_pool(name="a_w", bufs=1) as a_w,
            tc.tile_pool(name="a_sb", bufs=3) as a_sb,
            tc.tile_pool(name="a_ps", bufs=3, space="PSUM") as a_ps,
            tc.tile_pool(name="a_ps2", bufs=2, space="PSUM") as a_ps2,
        ):
            # --- k path first so the AllGather launches early ---
            wkT = a_w.tile([P, DC, D], f32, tag="wkT")
            transpose_weight("wk", wkT, a_sb, a_ps, f32r)
            kTo = a_w.tile([P, DC, B], f32, tag="kTo")
            transpose_rows(k_d, B, kTo, a_sb, a_ps, f32r)
            kpTo = a_w.tile([P, DC, B], f32, tag="kpTo")
            for jt in range(DC):
                pj = a_ps2.tile([P, B], f32, tag="pq")
                for dt in range(DC):
                    nc.tensor.matmul(pj[:],
                                     wkT[:, dt, jt * P:(jt + 1) * P].bitcast(f32r),
                                     kTo[:, dt, :].bitcast(f32r),
                                     start=(dt == 0), stop=(dt == DC - 1))
                nc.scalar.activation(kpTo[:, jt, :].bitcast(f32r), pj[:], IDENT,
                                     bias=kb_col[:, jt:jt + 1])
                nc.sync.dma_start(kp_in[jt * P:(jt + 1) * P, :], kpTo[:, jt, :])
            nc.gpsimd.collective_compute(
                "AllGather", mybir.AluOpType.bypass,
                replica_groups=[list(range(N_CORES))],
                ins=[kp_in[:].opt()], outs=[kp_all[:].opt()])

            # --- q path (overlaps the gather) ---
            wqT = a_w.tile([P, DC, D], f32, tag="wqT")
            transpose_weight("wq", wqT, a_sb, a_ps, f32r)
            qT = a_w.tile([P, DC, B], f32, tag="qT")
            transpose_rows(q_d, B, qT, a_sb, a_ps, f32r)
            for jt in range(DC):
                pj = a_ps2.tile([P, B], f32, tag="pq")
                for dt in range(DC):
                    nc.tensor.matmul(pj[:],
                                     wqT[:, dt, jt * P:(jt + 1) * P].bitcast(f32r),
                                     qT[:, dt, :].bitcast(f32r),
                                     start=(dt == 0), stop=(dt == DC - 1))
                nc.scalar.activation(qpT[:, jt, :].bitcast(f32r), pj[:], IDENT,
                                     bias=qb_col[:, jt:jt + 1])

        # ---- phase C: vp_own -> AllGather; woT -----------------------
        poolC_cm = tc.tile_pool(name="woT", bufs=1, side="right")   # woT: C..D
        poolC = poolC_cm.__enter__()
        woT = poolC.tile([P, DC, D], f16, tag="woT")

        with (
            tc.tile_pool(name="c_w", bufs=1) as c_w,
            tc.tile_pool(name="c_sb", bufs=3) as c_sb,
            tc.tile_pool(name="c_ps", bufs=3, space="PSUM") as c_ps,
            tc.tile_pool(name="c_ps2", bufs=2, space="PSUM") as c_ps2,
        ):
            wvT = c_w.tile([P, DC, D], f16, tag="wvT")
            transpose_weight("wv", wvT, c_sb, c_ps, f16)
            vTo = c_w.tile([P, DC, B], f16, tag="vTo")
            transpose_rows(v_d, B, vTo, c_sb, c_ps, f16)
            vpo = c_w.tile([P, IT, D], f16, tag="vpo")
            for lc in range(IT):
                for ms in range(MS):
                    pv = c_ps2.tile([P, FS], f32, tag="pv")
                    for dt in range(DC):
                        nc.tensor.matmul(pv[:], vTo[:, dt, lc * P:(lc + 1) * P],
                                         wvT[:, dt, ms * FS:(ms + 1) * FS],
                                         start=(dt == 0), stop=(dt == DC - 1))
                    nc.vector.tensor_add(vpo[:, lc, ms * FS:(ms + 1) * FS],
                                         pv[:], vb_bc[:, ms * FS:(ms + 1) * FS])
                nc.sync.dma_start(vp_in[lc * P:(lc + 1) * P, :], vpo[:, lc, :])
            nc.gpsimd.collective_compute(
                "AllGather", mybir.AluOpType.bypass,
                replica_groups=[list(range(N_CORES))],
                ins=[vp_in[:].opt()], outs=[vp_all[:].opt()])

            # wo transposes fill the gather wait
            transpose_weight("wo", woT, c_sb, c_ps, f16)

        # ---- phase B1: gathered kpT panels -> scores -----------------
        poolB_cm = tc.tile_pool(name="scores", bufs=1, side="right")
        poolB = poolB_cm.__enter__()
        scoresR = poolB.tile([P, IT, N], f32, tag="scores")

        with (
            tc.tile_pool(name="b_sb", bufs=2) as b_sb,
            tc.tile_pool(name="b_ps3", bufs=4, space="PSUM") as b_ps3,
        ):
            for lp in range(LPAN):
                kpP = b_sb.tile([P, DC, PW], f32, tag="kpP")
                for jt in range(DC):
                    eng = nc.sync if jt % 2 == 0 else nc.scalar
                    if _DIRECT_F32R:
                        eng.dma_start(
                            kpP[:, jt, :].bitcast(f32r),
                            kp_all[lp, jt * P:(jt + 1) * P, :].bitcast(f32r))
                    else:
                        eng.dma_start(kpP[:, jt, :],
                                      kp_all[lp, jt * P:(jt + 1) * P, :])
                if _DIRECT_F32R:
                    kpPr = kpP
                else:
                    kpPr = b_sb.tile([P, DC, PW], f32, tag="kpPr")
                    nc.vector.tensor_copy(kpPr[:].bitcast(f32r), kpP[:])
                for it in range(IT):
                    psc = b_ps3.tile([P, PW], f32, tag="ps")
                    for jt in range(DC):
                        nc.tensor.matmul(psc[:],
                                         qpT[:, jt, it * P:(it + 1) * P].bitcast(f32r),
                                         kpPr[:, jt, :].bitcast(f32r),
                                         start=(jt == 0), stop=(jt == DC - 1))
                    nc.vector.tensor_copy(scoresR[:, it, lp * PW:(lp + 1) * PW],
                                          psc[:])
        poolA_cm.__exit__(None, None, None)   # free qpT

        # ---- phase B2 + D: softmax, attnT, vp load, x1, out ----------
        poolBD_cm = tc.tile_pool(name="attnT", bufs=1)
        poolBD = poolBD_cm.__enter__()
        attnT = poolBD.tile([P, LT, B], f16, tag="attnT")
        vp = poolBD.tile([P, LT, D], f16, tag="vp")

        # vp: DMA gathered [N_CORES, B, D] -> [P, LT, D]
        for lc in range(LT):
            c, r = divmod(lc, IT)
            eng = nc.sync if lc % 2 == 0 else nc.scalar
            eng.dma_start(vp[:, lc, :], vp_all[c, r * P:(r + 1) * P, :])

        # merged per-it pipeline: softmax -> attnT -> x1 -> out proj
        with (
            tc.tile_pool(name="s_sb", bufs=2) as s_sb,
            tc.tile_pool(name="d_sb", bufs=1) as d_sb,
            tc.tile_pool(name="s_ps", bufs=2, space="PSUM") as s_ps,
            tc.tile_pool(name="d_ps", bufs=2, space="PSUM") as d_ps,
            tc.tile_pool(name="d_ps2", bufs=2, space="PSUM") as d_ps2,
        ):
            for it in range(IT):
                sc = scoresR[:, it, :]
                nmax = s_sb.tile([P, 1], f32, tag="nmax")
                nc.vector.reduce_max(nmax[:], sc, axis=mybir.AxisListType.X,
                                     negate=True)
                zsum = s_sb.tile([P, 1], f32, tag="zsum")
                nc.scalar.activation(sc, sc, EXP, bias=nmax[:], scale=1.0,
                                     accum_out=zsum[:])
                zinv = s_sb.tile([P, 1], f32, tag="zinv")
                nc.vector.reciprocal(zinv[:], zsum[:])
                nc.vector.tensor_scalar_mul(sc, sc, zinv[:])
                for lg in range(N // FS):
                    ab = s_sb.tile([P, FS], f16, tag="abf")
                    nc.scalar.activation(ab[:], sc[:, lg * FS:(lg + 1) * FS],
                                         COPY)
                    for li in range(FS // P):
                        lt = lg * (FS // P) + li
                        pt = s_ps.tile([P, P], f16, tag="tp16")
                        nc.tensor.transpose(pt[:], ab[:, li * P:(li + 1) * P],
                                            ident16[:])
                        nc.vector.tensor_copy(
                            attnT[:, lt, it * P:(it + 1) * P], pt[:])
                nc.sync.dma_start(a_out[it * P:(it + 1) * P, :], sc)

                x1b = d_sb.tile([P, D], f16, tag="x1b")
                for ms in range(MS):
                    px = d_ps.tile([P, FS], f32, tag="px")
                    for lt in range(LT):
                        nc.tensor.matmul(px[:], attnT[:, lt, it * P:(it + 1) * P],
                                         vp[:, lt, ms * FS:(ms + 1) * FS],
                                         start=(lt == 0), stop=(lt == LT - 1))
                    nc.scalar.activation(x1b[:, ms * FS:(ms + 1) * FS], px[:],
                                         COPY)
                x1T = d_sb.tile([P, DC, P], f16, tag="x1T")
                for mt in range(DC):
                    pt = d_ps.tile([P, P], f16, tag="tp16d")
                    nc.tensor.transpose(pt[:], x1b[:, mt * P:(mt + 1) * P],
                                        ident16[:])
                    nc.vector.tensor_copy(x1T[:, mt, :], pt[:])
                xo = d_sb.tile([P, D], f32, tag="xo")
                for cs in range(MS):
                    pxo = d_ps2.tile([P, FS], f32, tag="pxo")
                    for mt in range(DC):
                        nc.tensor.matmul(pxo[:], x1T[:, mt, :],
                                         woT[:, mt, cs * FS:(cs + 1) * FS],
                                         start=(mt == 0), stop=(mt == DC - 1))
                    nc.vector.tensor_add(xo[:, cs * FS:(cs + 1) * FS], pxo[:],
                                         ob_bc[:, cs * FS:(cs + 1) * FS])
                nc.scalar.dma_start(x_out[it * P:(it + 1) * P, :], xo[:])
        poolB_cm.__exit__(None, None, None)   # free scoresR

        poolBD_cm.__exit__(None, None, None)
        poolC_cm.__exit__(None, None, None)
        dram_cm.__exit__(None, None, None)
        const_cm.__exit__(None, None, None)

    nc.compile()
    return nc


_built = {}


def _get_nc(N=4096, D=1024):
    key = (N, D)
    if key not in _built:
        _built[key] = build(N, D)
    return _built[key]


def _make_in_maps(inputs):
    q = np.ascontiguousarray(np.asarray(inputs["q"], dtype=np.float32))
    k = np.ascontiguousarray(np.asarray(inputs["k"], dtype=np.float32))
    v = np.ascontiguousarray(np.asarray(inputs["v"], dtype=np.float32))
    N, D = k.shape
    B = N // N_CORES
    shared = {}
    for nm in ("wq", "wk", "wv", "wo"):
        shared[nm + "_w"] = np.ascontiguousarray(
            np.asarray(inputs[nm + "_w"], dtype=np.float32))
        shared[nm + "_b"] = np.ascontiguousarray(
            np.asarray(inputs[nm + "_b"], dtype=np.float32))
    return [dict(shared,
                 q=q[c * B:(c + 1) * B],
                 k=np.ascontiguousarray(k[c * B:(c + 1) * B]),
                 v=np.ascontiguousarray(v[c * B:(c + 1) * B]))
            for c in range(N_CORES)], N, D


def kernel(**inputs):
    in_maps, N, D = _make_in_maps(inputs)
    nc = _get_nc(N, D)
    res = run_bass_kernel_spmd(nc, in_maps, core_ids=list(range(N_CORES)))
    x = np.concatenate([res.results[c]["x_out"] for c in range(N_CORES)], axis=0)
    attn = np.concatenate([res.results[c]["attn_out"] for c in range(N_CORES)],
                          axis=0)
    return (x, attn)


# revision 19
# speedup vs baseline: 1.3746x; 1.0715x over previous
"""Multi-head-attention (single-head, no scaling) Bass kernel for 8 trn2 cores.

v2: distributed K/V projections + AllGather.

Sharding: core c owns q rows [c*B, (c+1)*B) AND k/v rows [c*B, (c+1)*B).
Weights replicated. Each core:
  A)  wk transpose (PE, f32 -> rounded f32r copyback), own-k transpose,
      kpT_own = (wk @ k_own.T + b) [D, B] f32r -> DRAM -> AllGather (CC queue)
      wq transpose, qT, qpT (overlaps the AllGather)
  C)  wv (fp16) transpose, own-v cast+transpose, vp_own [B, D] fp16
      (+bias) -> DRAM -> AllGather; wo (fp16) transpose
  B1) for each 512-panel: DMA gathered kpT panel, DVE re-round to f32r,
      scores matmuls -> resident [B, N] f32
  B2) per 128-row tile: softmax (DVE/ACT, fused exp+rowsum), DMA attn out,
      cast fp16 + PE transpose -> attnT
  Dv) DMA gathered vp -> SBUF fp16
  D)  x1 = attn @ vp, transpose, x = x1 @ wo.T + b -> DMA out
"""

import numpy as np

_DIRECT_F32R = True

try:
    import concourse.bass  # noqa: F401
except ImportError:
    import sys
    for _p in ("/opt/trn_rl_repo", "/root/.axon_site/_ro/trn_rl_repo"):
        if _p not in sys.path:
            sys.path.append(_p)

import concourse.bacc as bacc
import concourse.mybir as mybir
from concourse.bass_utils import run_bass_kernel_spmd
from concourse.tile import TileContext
from concourse.masks import make_identity

P = 128
N_CORES = 8

f32 = mybir.dt.float32
f32r = mybir.dt.float32r
f16 = mybir.dt.float16
EXP = mybir.ActivationFunctionType.Exp
COPY = mybir.ActivationFunctionType.Copy
IDENT = mybir.ActivationFunctionType.Identity


def build(N=4096, D=1024):
    B = N // N_CORES            # q/k/v rows per core
    DC = D // P                 # contraction chunks (d)
    IT = B // P                 # 128-row tiles per core slice
    PW = B                      # scores panel width == per-core slice
    LPAN = N // PW
    LT = N // P
    FS = min(512, D)
    MS = D // FS

    nc = bacc.Bacc("TRN2", target_bir_lowering=False, debug=False,
                   num_devices=N_CORES)

    q_d = nc.dram_tensor("q", [B, D], f32, kind="ExternalInput")
    k_d = nc.dram_tensor("k", [B, D], f32, kind="ExternalInput")
    v_d = nc.dram_tensor("v", [B, D], f32, kind="ExternalInput")
    w_d = {}
    b_d = {}
    for nm in ("wq", "wk", "wv", "wo"):
        w_d[nm] = nc.dram_tensor(nm + "_w", [D, D], f32, kind="ExternalInput")
        b_d[nm] = nc.dram_tensor(nm + "_b", [D], f32, kind="ExternalInput")
    x_out = nc.dram_tensor("x_out", [B, D], f32, kind="ExternalOutput")
    a_out = nc.dram_tensor("attn_out", [B, N], f32, kind="ExternalOutput")

    with TileContext(nc) as tc:
        const_cm = tc.tile_pool(name="const", bufs=1)
        const = const_cm.__enter__()
        dram_cm = tc.tile_pool(name="dram", bufs=1, space="DRAM")
        dram = dram_cm.__enter__()

        HB = B // 2             # column half of the kpT gather
        kp_in_h = [dram.tile([D, HB], f32, tag=f"kp_in{h}", name=f"kp_in{h}")
                   for h in range(2)]
        kp_all_h = [dram.tile([N_CORES, D, HB], f32, tag=f"kp_all{h}",
                                  name=f"kp_all{h}") for h in range(2)]
        vp_in = dram.tile([B, D], f16, tag="vp_in")
        vp_all = dram.tile([N_CORES, B, D], f16, tag="vp_all")
        warm_in = dram.tile([1, 32], f32, tag="warm_in")
        warm_out = dram.tile([N_CORES, 32], f32, tag="warm_out")

        # tiny warm-up collective: absorbs comm-init/rendezvous cost while
        # the input DMAs and weight transposes run
        warm_sb = const.tile([1, 32], f32, tag="warm")
        nc.gpsimd.memset(warm_sb[:], 0.0)
        nc.gpsimd.dma_start(warm_in[:], warm_sb[:])
        nc.gpsimd.collective_compute(
            "AllGather", mybir.AluOpType.bypass,
            replica_groups=[list(range(N_CORES))],
            ins=[warm_in[:].opt()], outs=[warm_out[:].opt()])

        ident32 = const.tile([P, P], f32, tag="id32")
        make_identity(nc, ident32[:])
        ident16 = const.tile([P, P], f16, tag="id16")
        make_identity(nc, ident16[:])

        qb_col = const.tile([P, DC], f32, tag="qbcol")
        kb_col = const.tile([P, DC], f32, tag="kbcol")
        for jt in range(DC):
            nc.sync.dma_start(qb_col[:, jt:jt + 1],
                              b_d["wq"][jt * P:(jt + 1) * P])
            nc.sync.dma_start(kb_col[:, jt:jt + 1],
                              b_d["wk"][jt * P:(jt + 1) * P])
        vb_bc = const.tile([P, D], f16, tag="vbbc")
        ob_bc = const.tile([P, D], f16, tag="obbc")
        with tc.tile_pool(name="btmp", bufs=2) as btmp:
            for bc_t, bnm in ((vb_bc, "wv"), (ob_bc, "wo")):
                tmp = btmp.tile([P, D], f32, tag="btmp")
                nc.sync.dma_start(
                    tmp[:],
                    b_d[bnm].rearrange("(o d) -> o d", o=1).partition_broadcast(P))
                nc.vector.tensor_copy(bc_t[:], tmp[:])

        def transpose_weight(nm, wT, sb, ps, rnd):
            """DMA weight natural, PE-transpose 128x128 blocks into wT."""
            for jt in range(DC):
                wnat = sb.tile([P, D], f32, tag="wnat")
                nc.sync.dma_start(wnat[:], w_d[nm][jt * P:(jt + 1) * P, :])
                for dt in range(DC):
                    pt = ps.tile([P, P], f32, tag="tpw")
                    nc.tensor.transpose(pt[:], wnat[:, dt * P:(dt + 1) * P],
                                        ident32[:])
                    dst = wT[:, dt, jt * P:(jt + 1) * P]
                    if rnd is f32r:
                        nc.scalar.activation(dst.bitcast(f32r), pt[:], COPY)
                    else:
                        nc.scalar.activation(dst, pt[:], COPY)

        def transpose_rows(src_d, rows, dst, sb, ps, dt_out):
            """DMA rows of src_d, transpose into dst [P, DC, rows]."""
            for rt in range(rows // P):
                nat = sb.tile([P, D], f32, tag="wnat")
                nc.sync.dma_start(nat[:], src_d[rt * P:(rt + 1) * P, :])
                if dt_out is f16:
                    c16 = sb.tile([P, D], f16, tag="nat16")
                    nc.vector.tensor_copy(c16[:], nat[:])
                    for dt in range(DC):
                        pt = ps.tile([P, P], f16, tag="tp16")
                        nc.tensor.transpose(pt[:], c16[:, dt * P:(dt + 1) * P],
                                            ident16[:])
                        nc.vector.tensor_copy(dst[:, dt, rt * P:(rt + 1) * P],
                                              pt[:])
                else:
                    for dt in range(DC):
                        pt = ps.tile([P, P], f32, tag="tpw")
                        nc.tensor.transpose(pt[:], nat[:, dt * P:(dt + 1) * P],
                                            ident32[:])
                        nc.vector.tensor_copy(
                            dst[:, dt, rt * P:(rt + 1) * P].bitcast(f32r), pt[:])

        # ---- phase A: kpT_own -> AllGather; qpT ----------------------
        poolA_cm = tc.tile_pool(name="qpT", bufs=1)   # qpT: A..B1
        poolA = poolA_cm.__enter__()
        qpT = poolA.tile([P, DC, B], f32, tag="qpT")

        with (
            tc.tile_pool(name="a_w", bufs=1) as a_w,
            tc.tile_pool(name="a_sb", bufs=3) as a_sb,
            tc.tile_pool(name="a_ps", bufs=3, space="PSUM") as a_ps,
            tc.tile_pool(name="a_ps2", bufs=2, space="PSUM") as a_ps2,
        ):
            # --- k path first so the AllGather launches early ---
            wkT = a_w.tile([P, DC, D], f32, tag="wkT")
            transpose_weight("wk", wkT, a_sb, a_ps, f32r)
            kTo = a_w.tile([P, DC, B], f32, tag="kTo")
            transpose_rows(k_d, B, kTo, a_sb, a_ps, f32r)
            kpTo = a_w.tile([P, DC, B], f32, tag="kpTo")
            for jt in range(DC):
                pj = a_ps2.tile([P, B], f32, tag="pq")
                for dt in range(DC):
                    nc.tensor.matmul(pj[:],
                                     wkT[:, dt, jt * P:(jt + 1) * P].bitcast(f32r),
                                     kTo[:, dt, :].bitcast(f32r),
                                     start=(dt == 0), stop=(dt == DC - 1))
                nc.scalar.activation(kpTo[:, jt, :].bitcast(f32r), pj[:], IDENT,
                                     bias=kb_col[:, jt:jt + 1])
                for h in range(2):
                    nc.sync.dma_start(kp_in_h[h][jt * P:(jt + 1) * P, :],
                                      kpTo[:, jt, h * HB:(h + 1) * HB])
            for h in range(2):
                nc.gpsimd.collective_compute(
                    "AllGather", mybir.AluOpType.bypass,
                    replica_groups=[list(range(N_CORES))],
                    ins=[kp_in_h[h][:].opt()], outs=[kp_all_h[h][:].opt()])

            # --- q path (overlaps the gather) ---
            wqT = a_w.tile([P, DC, D], f32, tag="wqT")
            transpose_weight("wq", wqT, a_sb, a_ps, f32r)
            qT = a_w.tile([P, DC, B], f32, tag="qT")
            transpose_rows(q_d, B, qT, a_sb, a_ps, f32r)
            for jt in range(DC):
                pj = a_ps2.tile([P, B], f32, tag="pq")
                for dt in range(DC):
                    nc.tensor.matmul(pj[:],
                                     wqT[:, dt, jt * P:(jt + 1) * P].bitcast(f32r),
                                     qT[:, dt, :].bitcast(f32r),
                                     start=(dt == 0), stop=(dt == DC - 1))
                nc.scalar.activation(qpT[:, jt, :].bitcast(f32r), pj[:], IDENT,
                                     bias=qb_col[:, jt:jt + 1])

        # ---- phase C: vp_own -> AllGather; woT -----------------------
        poolC_cm = tc.tile_pool(name="woT", bufs=1, side="right")   # woT: C..D
        poolC = poolC_cm.__enter__()
        woT = poolC.tile([P, DC, D], f16, tag="woT")

        with (
            tc.tile_pool(name="c_w", bufs=1) as c_w,
            tc.tile_pool(name="c_sb", bufs=3) as c_sb,
            tc.tile_pool(name="c_ps", bufs=3, space="PSUM") as c_ps,
            tc.tile_pool(name="c_ps2", bufs=2, space="PSUM") as c_ps2,
        ):
            wvT = c_w.tile([P, DC, D], f16, tag="wvT")
            transpose_weight("wv", wvT, c_sb, c_ps, f16)
            vTo = c_w.tile([P, DC, B], f16, tag="vTo")
            transpose_rows(v_d, B, vTo, c_sb, c_ps, f16)
            vpo = c_w.tile([P, IT, D], f16, tag="vpo")
            for lc in range(IT):
                for ms in range(MS):
                    pv = c_ps2.tile([P, FS], f32, tag="pv")
                    for dt in range(DC):
                        nc.tensor.matmul(pv[:], vTo[:, dt, lc * P:(lc + 1) * P],
                                         wvT[:, dt, ms * FS:(ms + 1) * FS],
                                         start=(dt == 0), stop=(dt == DC - 1))
                    nc.vector.tensor_add(vpo[:, lc, ms * FS:(ms + 1) * FS],
                                         pv[:], vb_bc[:, ms * FS:(ms + 1) * FS])
                nc.sync.dma_start(vp_in[lc * P:(lc + 1) * P, :], vpo[:, lc, :])
            nc.gpsimd.collective_compute(
                "AllGather", mybir.AluOpType.bypass,
                replica_groups=[list(range(N_CORES))],
                ins=[vp_in[:].opt()], outs=[vp_all[:].opt()])

            # wo transposes fill the gather wait
            transpose_weight("wo", woT, c_sb, c_ps, f16)

        # ---- phase B1: gathered kpT panels -> scores -----------------
        poolB_cm = tc.tile_pool(name="scores", bufs=1, side="right")
        poolB = poolB_cm.__enter__()
        scoresR = poolB.tile([P, IT, N], f32, tag="scores")

        with (
            tc.tile_pool(name="b_sb", bufs=4) as b_sb,
            tc.tile_pool(name="b_ps3", bufs=4, space="PSUM") as b_ps3,
        ):
            for h in range(2):
                for lp in range(LPAN):
                    kpP = b_sb.tile([P, DC, HB], f32, tag="kpP")
                    for jt in range(DC):
                        eng = nc.sync if jt % 2 == 0 else nc.scalar
                        eng.dma_start(
                            kpP[:, jt, :].bitcast(f32r),
                            kp_all_h[h][lp, jt * P:(jt + 1) * P, :].bitcast(f32r))
                    for it in range(IT):
                        psc = b_ps3.tile([P, HB], f32, tag="ps")
                        for jt in range(DC):
                            nc.tensor.matmul(
                                psc[:],
                                qpT[:, jt, it * P:(it + 1) * P].bitcast(f32r),
                                kpP[:, jt, :].bitcast(f32r),
                                start=(jt == 0), stop=(jt == DC - 1))
                        nc.vector.tensor_copy(
                            scoresR[:, it, lp * PW + h * HB:lp * PW + (h + 1) * HB],
                            psc[:])
        poolA_cm.__exit__(None, None, None)   # free qpT

        # ---- phase B2 + D: softmax, attnT, vp load, x1, out ----------
        poolBD_cm = tc.tile_pool(name="attnT", bufs=1)
        poolBD = poolBD_cm.__enter__()
        attnT = poolBD.tile([P, LT, B], f16, tag="attnT")
        vp = poolBD.tile([P, LT, D], f16, tag="vp")

        # vp: DMA gathered [N_CORES, B, D] -> [P, LT, D]
        for lc in range(LT):
            c, r = divmod(lc, IT)
            eng = nc.sync if lc % 2 == 0 else nc.scalar
            eng.dma_start(vp[:, lc, :], vp_all[c, r * P:(r + 1) * P, :])

        # merged per-it pipeline: softmax -> attnT -> x1 -> out proj
        with (
            tc.tile_pool(name="s_sb", bufs=2) as s_sb,
            tc.tile_pool(name="d_sb", bufs=1) as d_sb,
            tc.tile_pool(name="s_ps", bufs=2, space="PSUM") as s_ps,
            tc.tile_pool(name="d_ps", bufs=2, space="PSUM") as d_ps,
            tc.tile_pool(name="d_ps2", bufs=2, space="PSUM") as d_ps2,
        ):
            for it in range(IT):
                sc = scoresR[:, it, :]
                nmax = s_sb.tile([P, 1], f32, tag="nmax")
                nc.vector.reduce_max(nmax[:], sc, axis=mybir.AxisListType.X,
                                     negate=True)
                zsum = s_sb.tile([P, 1], f32, tag="zsum")
                nc.scalar.activation(sc, sc, EXP, bias=nmax[:], scale=1.0,
                                     accum_out=zsum[:])
                zinv = s_sb.tile([P, 1], f32, tag="zinv")
                nc.vector.reciprocal(zinv[:], zsum[:])
                nc.vector.tensor_scalar_mul(sc, sc, zinv[:])
                for lg in range(N // FS):
                    ab = s_sb.tile([P, FS], f16, tag="abf")
                    nc.scalar.activation(ab[:], sc[:, lg * FS:(lg + 1) * FS],
                                         COPY)
                    for li in range(FS // P):
                        lt = lg * (FS // P) + li
                        pt = s_ps.tile([P, P], f16, tag="tp16")
                        nc.tensor.transpose(pt[:], ab[:, li * P:(li + 1) * P],
                                            ident16[:])
                        nc.vector.tensor_copy(
                            attnT[:, lt, it * P:(it + 1) * P], pt[:])
                nc.sync.dma_start(a_out[it * P:(it + 1) * P, :], sc)

                x1b = d_sb.tile([P, D], f16, tag="x1b")
                for ms in range(MS):
                    px = d_ps.tile([P, FS], f32, tag="px")
                    for lt in range(LT):
                        nc.tensor.matmul(px[:], attnT[:, lt, it * P:(it + 1) * P],
                                         vp[:, lt, ms * FS:(ms + 1) * FS],
                                         start=(lt == 0), stop=(lt == LT - 1))
                    nc.scalar.activation(x1b[:, ms * FS:(ms + 1) * FS], px[:],
                                         COPY)
                x1T = d_sb.tile([P, DC, P], f16, tag="x1T")
                for mt in range(DC):
                    pt = d_ps.tile([P, P], f16, tag="tp16d")
                    nc.tensor.transpose(pt[:], x1b[:, mt * P:(mt + 1) * P],
                                        ident16[:])
                    nc.vector.tensor_copy(x1T[:, mt, :], pt[:])
                xo = d_sb.tile([P, D], f32, tag="xo")
                for cs in range(MS):
                    pxo = d_ps2.tile([P, FS], f32, tag="pxo")
                    for mt in range(DC):
                        nc.tensor.matmul(pxo[:], x1T[:, mt, :],
                                         woT[:, mt, cs * FS:(cs + 1) * FS],
                                         start=(mt == 0), stop=(mt == DC - 1))
                    nc.vector.tensor_add(xo[:, cs * FS:(cs + 1) * FS], pxo[:],
                                         ob_bc[:, cs * FS:(cs + 1) * FS])
                nc.scalar.dma_start(x_out[it * P:(it + 1) * P, :], xo[:])
        poolB_cm.__exit__(None, None, None)   # free scoresR

        poolBD_cm.__exit__(None, None, None)
        poolC_cm.__exit__(None, None, None)
        dram_cm.__exit__(None, None, None)
        const_cm.__exit__(None, None, None)

    nc.compile()
    return nc


_built = {}


def _get_nc(N=4096, D=1024):
    key = (N, D)
    if key not in _built:
        _built[key] = build(N, D)
    return _built[key]


def _make_in_maps(inputs):
    q = np.ascontiguousarray(np.asarray(inputs["q"], dtype=np.float32))
    k = np.ascontiguousarray(np.asarray(inputs["k"], dtype=np.float32))
    v = np.ascontiguousarray(np.asarray(inputs["v"], dtype=np.float32))
    N, D = k.shape
    B = N // N_CORES
    shared = {}
    for nm in ("wq", "wk", "wv", "wo"):
        shared[nm + "_w"] = np.ascontiguousarray(
            np.asarray(inputs[nm + "_w"], dtype=np.float32))
        shared[nm + "_b"] = np.ascontiguousarray(
            np.asarray(inputs[nm + "_b"], dtype=np.float32))
    return [dict(shared,
                 q=q[c * B:(c + 1) * B],
                 k=np.ascontiguousarray(k[c * B:(c + 1) * B]),
                 v=np.ascontiguousarray(v[c * B:(c + 1) * B]))
            for c in range(N_CORES)], N, D


def kernel(**inputs):
    in_maps, N, D = _make_in_maps(inputs)
    nc = _get_nc(N, D)
    res = run_bass_kernel_spmd(nc, in_maps, core_ids=list(range(N_CORES)))
    x = np.concatenate([res.results[c]["x_out"] for c in range(N_CORES)], axis=0)
    attn = np.concatenate([res.results[c]["attn_out"] for c in range(N_CORES)],
                          axis=0)
    return (x, attn)
